# revision 1
# baseline (speedup 1.0000x reference)
"""Trainium2 Bass kernel for nn_CMDPEncoder (VQ codebook quantize + random
batch-mix dequantize + DP noise).

Reference semantics:
    dots = einsum('bsd,vd->bsv', base, codebook)
    qi   = argmin_v(csq[v] - 2*dots)                  # [B,S]
    codes[b,s,j] = qi[rand_idx[b,s,j], s]
    out  = mean_j codebook[codes] + 0.1*noise

Sharding: split the sequence dim S across the 8 cores (64 positions each).
The rand_idx mixing crosses only the batch dim at fixed s, so with S-sharding
every core's mixing is fully local (no collectives).  Tokens are laid out
s-major (t = s_local*16 + b) so each 128-token tile holds 8 complete
s-groups of 16 batches, and the mix becomes a block-diagonal [128,128]
matmul with host-precomputed weights (counts/4 from rand_idx).

Scoring runs on the tensor engine: scores = 2*dots - (csq-768), with the
csq term folded in as an extra K=2 contraction chunk in fp16 hi/lo pairs
(exact to ~6e-5; the min top-2 score gap on this data is ~2.2e-3).

Score matmul variants (VARIANT):
  fp32    - plain fp32 matmuls (4 cycles/row). Safe, slowest.
  fp16x3  - 3-term Dekker split 2x*c = xh*ch + xh*cl + xl*ch in fp16
            (1 cycle/row, 18 chunks). Error ~1e-5, safe, ~25% faster.
  fp32r   - single-pass float32r (1 cycle/row, 6 chunks) + exact top-2
            rescore/fixup on DVE. fp32r alone has ~2e-2 max dot error,
            so the top-2 candidates are rescored with exact fp32 dots and
            the winner picked from those. Fastest.

Argmax via DVE max/max_index, dequantize via gpsimd indirect DMA gather of
codebook rows, mix via a second matmul, noise added during the PSUM drain.
"""

import os
import sys

for p in ("/opt/trn_rl_repo",):
    if p not in sys.path:
        sys.path.insert(0, p)

import numpy as np

import concourse.bacc as bacc
import concourse.bass as bass
import concourse.mybir as mybir
import concourse.tile as tile
from concourse.bass_utils import run_bass_kernel_spmd

B, S, D, V, K = 16, 512, 768, 4096, 4
N_CORES = 8
SS = S // N_CORES            # 64 sequence positions per core
T = SS * B                   # 1024 tokens per core, t = s_local*16 + b
TT = T // 128                # 8 token tiles per core
KC = D // 128                # 6 contraction chunks
NV = V // 512                # 8 V-tiles
DP_EPSILON = 0.1
CSQ_CENTER = 768.0
DE = 776                     # padded cb_ext row: 768 cb + 1 csq + 7 pad

F32 = mybir.dt.float32
F32R = mybir.dt.float32r
F16 = mybir.dt.float16
BF16 = mybir.dt.bfloat16
U32 = mybir.dt.uint32
I32 = mybir.dt.int32

VARIANT = os.environ.get("CMDP_VARIANT", "bf16fix")

_CACHED = {}


def _is_fixup(variant):
    return variant.endswith("fix")


def _base(variant):
    return variant[:-3] if variant.endswith("fix") else variant


def _score_terms(variant):
    """[(lhs_tensor_name, rhs_tensor_name, dtype)] for the 6-chunk terms."""
    base = _base(variant)
    if base == "fp32":
        return [("xT", "cbT", F32)]
    if base == "fp16x3":
        return [("xTh", "cbTh", F16), ("xTh", "cbTl", F16), ("xTl", "cbTh", F16)]
    if base == "fp32r":
        return [("xT", "cbT", F32R)]
    if base == "bf16":
        return [("xTb", "cbTb", BF16)]
    raise ValueError(variant)


def _build_nc(variant):
    fixup = _is_fixup(variant)
    terms = _score_terms(variant)
    lhs_names = sorted({t[0] for t in terms})
    rhs_names = sorted({t[1] for t in terms})

    nc = bacc.Bacc("TRN2", target_bir_lowering=False, debug=False,
                   num_devices=N_CORES)

    lhs_d = {n: nc.dram_tensor(n, [128, KC * T],
                               [t[2] for t in terms if t[0] == n][0],
                               kind="ExternalInput") for n in lhs_names}
    rhs_d = {n: nc.dram_tensor(n, [128, KC * V],
                               [t[2] for t in terms if t[1] == n][0],
                               kind="ExternalInput") for n in rhs_names}
    cbe_d = nc.dram_tensor("cbe", [V, DE], F32, kind="ExternalInput")
    csqL_d = nc.dram_tensor("csqL", [2, T], F16, kind="ExternalInput")
    csqR_d = nc.dram_tensor("csqR", [2, V], F16, kind="ExternalInput")
    w_d = nc.dram_tensor("w", [128, TT * 128], F32, kind="ExternalInput")
    noise_d = nc.dram_tensor("noise", [T, D], F32, kind="ExternalInput")
    if fixup:
        xn_d = nc.dram_tensor("xn", [128, TT * D], F32, kind="ExternalInput")
    out_d = nc.dram_tensor("out", [T, D], F32, kind="ExternalOutput")

    with tile.TileContext(nc) as tc:
        with (
            tc.tile_pool(name="big", bufs=1) as big,
            tc.tile_pool(name="work", bufs=2) as work,
            tc.tile_pool(name="sc", bufs=3) as sc_pool,
            tc.tile_pool(name="ypool", bufs=4) as ypool,
            tc.tile_pool(name="io", bufs=3) as io,
            tc.tile_pool(name="ps_s", bufs=6, space="PSUM") as ps_s,
            tc.tile_pool(name="ps_m", bufs=1, space="PSUM") as ps_m,
        ):
            # host pre-tiles inputs to [128, ...]; stream order is chosen so
            # the PE can issue its first matmul ~3us in: xt tile 0, then the
            # codebook in v-blocks (one per 512-wide V-tile), then per-tile
            # xn/xt interleaved.
            XTW = KC * 128   # xt columns per token tile
            VBW = KC * 512   # codebook columns per v-block
            # separate tiles per v-block / token-tile so Tile's dependency
            # tracking gates each matmul on exactly the DMA it needs
            lhs_t = {n: [] for n in lhs_d}
            rhs_t = {n: [] for n in rhs_d}
            xn_t = []
            csql = big.tile([2, T], F16)
            csqr = big.tile([2, V], F16)
            nc.sync.dma_start(csql[:], csqL_d.ap())
            nc.sync.dma_start(csqr[:], csqR_d.ap())
            for n, d in lhs_d.items():
                tl = big.tile([128, XTW], d.dtype, tag=f"{n}0")
                nc.sync.dma_start(tl[:], d.ap()[:, 0:XTW])
                lhs_t[n].append(tl)
            for v in range(NV):
                for n, d in rhs_d.items():
                    tl = big.tile([128, VBW], d.dtype, tag=f"{n}v{v}")
                    nc.sync.dma_start(tl[:], d.ap()[:, v * VBW:(v + 1) * VBW])
                    rhs_t[n].append(tl)
            if fixup:
                tl = big.tile([128, D], F32, tag="xn0")
                nc.sync.dma_start(tl[:], xn_d.ap()[:, 0:D])
                xn_t.append(tl)
            for t in range(1, TT):
                for n, d in lhs_d.items():
                    tl = big.tile([128, XTW], d.dtype, tag=f"{n}{t}")
                    nc.sync.dma_start(tl[:], d.ap()[:, t * XTW:(t + 1) * XTW])
                    lhs_t[n].append(tl)
                if fixup:
                    tl = big.tile([128, D], F32, tag=f"xn{t}")
                    nc.sync.dma_start(tl[:], xn_d.ap()[:, t * D:(t + 1) * D])
                    xn_t.append(tl)
            w = big.tile([128, TT * 128], F32)
            nc.sync.dma_start(w[:], w_d.ap())
            # last tile's noise pre-staged in SBUF: its add runs on the (by
            # then idle) DVE instead of the ACT-drain -> accum-DMA chain
            nzlast = big.tile([128, 2 * D], F32)
            for a in range(2):
                tt_ = TT - 2 + a
                nc.sync.dma_start(nzlast[:, a * D:(a + 1) * D],
                                  noise_d.ap()[tt_ * 128:(tt_ + 1) * 128, :])

            def emit_scoring(t):
                tsl = slice(t * 128, (t + 1) * 128)
                scores = sc_pool.tile([128, V], F32, tag="scores")
                for v in range(NV):
                    vsl = slice(v * 512, (v + 1) * 512)
                    ps = ps_s.tile([128, 512], F32, tag="ps_score")
                    i = 0
                    for (ln, rn, _dt) in terms:
                        for k in range(KC):
                            nc.tensor.matmul(
                                ps[:],
                                lhs_t[ln][t][:, k * 128:(k + 1) * 128],
                                rhs_t[rn][v][:, k * 512:(k + 1) * 512],
                                start=(i == 0), stop=False)
                            i += 1
                    nc.tensor.matmul(ps[:], csql[:, tsl], csqr[:, vsl],
                                     start=False, stop=True)
                    nc.scalar.copy(out=scores[:, vsl], in_=ps[:])
                return scores

            def emit_scan_fixup(t, scores):
                """argmax (+ exact top-2 rescore) -> gather y rows."""
                tsl = slice(t * 128, (t + 1) * 128)
                mx = work.tile([128, 8], F32, tag="mx")
                idx = work.tile([128, 8], U32, tag="idx")
                nc.vector.max(mx[:], scores[:])
                nc.vector.max_index(idx[:], mx[:], scores[:])

                if not fixup:
                    idx32 = work.tile([128, 1], I32, tag="idx32")
                    nc.vector.tensor_copy(idx32[:], idx[:, 0:1])
                else:
                    # exact top-2 rescore: s_j = csq[cand_j] - 2*x.cb[cand_j]
                    xn = xn_t[t][:]
                    cand = []
                    for j in range(2):
                        cj = work.tile([128, 1], I32, tag=f"cand{j}")
                        nc.vector.tensor_copy(cj[:], idx[:, j:j + 1])
                        cand.append(cj)
                    sj = []
                    for j in range(2):
                        g = work.tile([128, DE], F32, tag=f"g{j}")
                        nc.gpsimd.indirect_dma_start(
                            out=g[:], out_offset=None, in_=cbe_d.ap(),
                            in_offset=bass.IndirectOffsetOnAxis(
                                ap=cand[j][:, :1], axis=0))
                        # NB: tensor_tensor_reduce hard-faults TRN2 here;
                        # scalar_tensor_tensor with accum_out does not.
                        tmp = work.tile([128, D], F32, tag="rescore_tmp")
                        dj = work.tile([128, 1], F32, tag=f"d{j}")
                        nc.vector.scalar_tensor_tensor(
                            out=tmp[:], in0=xn, scalar=1.0, in1=g[:, 0:D],
                            op0=mybir.AluOpType.bypass,
                            op1=mybir.AluOpType.mult, accum_out=dj[:])
                        s = work.tile([128, 1], F32, tag=f"s{j}")
                        # s = (dj * -2) + csq_cand
                        nc.vector.scalar_tensor_tensor(
                            out=s[:], in0=dj[:], scalar=-2.0, in1=g[:, D:D + 1],
                            op0=mybir.AluOpType.mult, op1=mybir.AluOpType.add)
                        sj.append(s)
                    flip = work.tile([128, 1], I32, tag="flip")
                    nc.vector.tensor_tensor(out=flip[:], in0=sj[1][:],
                                            in1=sj[0][:],
                                            op=mybir.AluOpType.is_lt)
                    idx32 = work.tile([128, 1], I32, tag="idx32")
                    nc.vector.tensor_copy(idx32[:], cand[0][:])
                    nc.vector.copy_predicated(idx32[:], flip[:], cand[1][:])

                y = ypool.tile([128, DE], F32, tag="y")
                nc.gpsimd.indirect_dma_start(
                    out=y[:], out_offset=None, in_=cbe_d.ap(),
                    in_offset=bass.IndirectOffsetOnAxis(ap=idx32[:, :1], axis=0))
                return y

            def emit_output(t, y):
                """mix matmul -> ACT drain -> noise accum-DMA -> store."""
                tsl = slice(t * 128, (t + 1) * 128)
                pm = ps_m.tile([128, D], F32, tag="pm")
                nc.tensor.matmul(pm[:, 0:512], w[:, tsl], y[:, 0:512],
                                 start=True, stop=True)
                nc.tensor.matmul(pm[:, 512:D], w[:, tsl], y[:, 512:D],
                                 start=True, stop=True)
                ob = io.tile([128, D], F32, tag="out")
                if t >= TT - 2:
                    nz = nzlast[:, (t - (TT - 2)) * D:(t - (TT - 2) + 1) * D]
                    nc.vector.tensor_add(ob[:], pm[:], nz)
                else:
                    nc.scalar.copy(out=ob[:], in_=pm[:])
                    # add DP noise inline in the DMA (SWDGE accumulate)
                    nc.gpsimd.dma_start(out=ob[:], in_=noise_d.ap()[tsl, :],
                                        accum_op=mybir.AluOpType.add)
                nc.sync.dma_start(out_d.ap()[tsl, :], ob[:])

            # 2-deep software pipeline: PE's instruction stream is
            # score(0) score(1) score(2) mix(0) score(3) mix(1) ... so the
            # scan/fixup/gather chain of tile t overlaps scoring of t+1/t+2
            # and the PE never stalls on it.
            PIPE = 3
            pending = []
            for t in range(TT):
                scores = emit_scoring(t)
                y = emit_scan_fixup(t, scores)
                pending.append((t, y))
                if len(pending) > PIPE:
                    emit_output(*pending.pop(0))
            for item in pending:
                emit_output(*item)

    nc.compile()
    return nc


def _prep_inputs(variant, base_embeddings, codebook, rand_idx, noise):
    """Build the 8 per-core input maps (all host-side numpy)."""
    fixup = _is_fixup(variant)
    x = np.ascontiguousarray(base_embeddings, dtype=np.float32)
    cb = np.ascontiguousarray(codebook, dtype=np.float32)
    ridx = np.asarray(rand_idx)
    nz = np.asarray(noise, dtype=np.float32)

    csq = (cb * cb).sum(-1, dtype=np.float32)              # [V]
    cbe = np.zeros((V, DE), np.float32)
    cbe[:, :D] = cb
    cbe[:, D] = csq
    csqc = (csq - CSQ_CENTER).astype(np.float32)
    r1 = csqc.astype(np.float16)
    r2 = (csqc - r1.astype(np.float32)).astype(np.float16)
    csqR = np.ascontiguousarray(np.stack([r1, r2]))        # [2, V] fp16
    csqL = np.full((2, T), -1.0, np.float16)

    shared = {"cbe": cbe, "csqL": csqL, "csqR": csqR}
    # pre-tile [D, V] -> [128, (v, k, 512)] v-block-major layout
    cbT = cb.T.reshape(KC, 128, NV, 512).transpose(1, 2, 0, 3).reshape(128, KC * V)
    cbT = np.ascontiguousarray(cbT)
    base = _base(variant)
    if base in ("fp32", "fp32r"):
        shared["cbT"] = cbT
    elif base == "bf16":
        import ml_dtypes
        shared["cbTb"] = cbT.astype(ml_dtypes.bfloat16)
    elif base == "fp16x3":
        cbh = cbT.astype(np.float16)
        cbl = (cbT - cbh.astype(np.float32)).astype(np.float16)
        shared["cbTh"] = cbh
        shared["cbTl"] = cbl

    in_maps = []
    for c in range(N_CORES):
        ssl = slice(c * SS, (c + 1) * SS)
        # tokens t = s_local*16 + b
        xc = x[:, ssl, :].transpose(1, 0, 2).reshape(T, D)
        xT2 = (2.0 * xc).T                                 # [D, T] fp32
        # pre-tile [D, T] -> [128, (t, k, 128)] tile-major layout
        xT2 = np.ascontiguousarray(
            xT2.reshape(KC, 128, TT, 128).transpose(1, 2, 0, 3).reshape(128, KC * T))
        nzc = np.ascontiguousarray(
            DP_EPSILON * nz[:, ssl, :].transpose(1, 0, 2).reshape(T, D))
        rc = ridx[:, ssl, :]                               # [B, SS, K]
        wm = np.zeros((TT, 128, 128), np.float32)
        for tt in range(TT):
            for g in range(8):
                s_local = tt * 8 + g
                r = rc[:, s_local, :]                      # [B, K] in [0,B)
                cnt = np.zeros((B, B), np.float32)         # [dst=b, src]
                for bdst in range(B):
                    np.add.at(cnt[bdst], r[bdst], 1.0)
                wm[tt, g * 16:(g + 1) * 16, g * 16:(g + 1) * 16] = cnt.T / K
        wm_t = np.ascontiguousarray(
            wm.transpose(1, 0, 2).reshape(128, TT * 128))
        m = {"w": wm_t, "noise": nzc, **shared}
        if base in ("fp32", "fp32r"):
            m["xT"] = xT2
        elif base == "bf16":
            import ml_dtypes
            m["xTb"] = xT2.astype(ml_dtypes.bfloat16)
        elif base == "fp16x3":
            xh = xT2.astype(np.float16)
            xl = (xT2 - xh.astype(np.float32)).astype(np.float16)
            m["xTh"] = xh
            m["xTl"] = xl
        if fixup:
            m["xn"] = np.ascontiguousarray(
                xc.reshape(TT, 128, D).transpose(1, 0, 2).reshape(128, TT * D))
        in_maps.append(m)
    return in_maps


def kernel(base_embeddings, codebook, rand_idx, noise, _results_out=None):
    variant = VARIANT
    if variant not in _CACHED:
        _CACHED[variant] = _build_nc(variant)
    nc = _CACHED[variant]
    in_maps = _prep_inputs(variant, base_embeddings, codebook, rand_idx, noise)
    res = run_bass_kernel_spmd(nc, in_maps, list(range(N_CORES)))
    if _results_out is not None:
        _results_out.append(res)
    outs = []
    for c in range(N_CORES):
        oc = res.results[c]["out"].reshape(SS, B, D).transpose(1, 0, 2)
        outs.append(oc)
    return np.ascontiguousarray(np.concatenate(outs, axis=1))



# revision 4
# speedup vs baseline: 1.0767x; 1.0767x over previous
"""Trainium2 Bass kernel for nn_CMDPEncoder (VQ codebook quantize + random
batch-mix dequantize + DP noise).

Reference semantics:
    dots = einsum('bsd,vd->bsv', base, codebook)
    qi   = argmin_v(csq[v] - 2*dots)                  # [B,S]
    codes[b,s,j] = qi[rand_idx[b,s,j], s]
    out  = mean_j codebook[codes] + 0.1*noise

Sharding: split the sequence dim S across the 8 cores (64 positions each).
The rand_idx mixing crosses only the batch dim at fixed s, so with S-sharding
every core's mixing is fully local (no collectives).  Tokens are laid out
s-major (t = s_local*16 + b) so each 128-token tile holds 8 complete
s-groups of 16 batches, and the mix becomes a block-diagonal [128,128]
matmul with host-precomputed weights (counts/4 from rand_idx).

v3 design:
  - scores accumulate in [128,1024] (2-bank) PSUM tiles; csq bias comes in
    via ACT-engine PSUM prefill (fp32 exact) + all-start=False matmuls.
    EXCEPT tile 0, which opens each PSUM bank the classic way
    (start=True + a 2-row fp16-pair csq matmul): a start=False matmul on a
    never-opened bank does not accumulate correctly on HW (v2 post-mortem:
    only tile-0 tokens were wrong).
  - prefills are issued 2 blocks ahead so the in-order ACT engine finishes
    them under the previous blocks' matmuls instead of stalling the PE.
  - mix matmul in bf16 (w is exact in bf16; y gathered from a bf16 copy of
    the codebook).
  - exact top-2 rescore dots run on GpSimd (DVE is co-critical with PE).
  - last tile scans per-1024-block incrementally (overlapped under its own
    scoring) + a tiny merge, shortening the serial tail.
"""

import os
import sys

for p in ("/opt/trn_rl_repo",):
    if p not in sys.path:
        sys.path.insert(0, p)

import numpy as np

import concourse.bacc as bacc
import concourse.bass as bass
import concourse.mybir as mybir
import concourse.tile as tile
from concourse.bass_utils import run_bass_kernel_spmd

B, S, D, V, K = 16, 512, 768, 4096, 4
N_CORES = 8
SS = S // N_CORES            # 64 sequence positions per core
T = SS * B                   # 1024 tokens per core, t = s_local*16 + b
TT = T // 128                # 8 token tiles per core
KC = D // 128                # 6 contraction chunks
NV = V // 512                # 8 V-tiles
NVP = V // 1024              # 4 V-pairs (2-bank PSUM tiles)
NBLK = TT * NVP              # 32 scoring blocks
DP_EPSILON = 0.1
CSQ_CENTER = 768.0
DE = 776                     # padded cb_ext row: 768 cb + 1 csq + 7 pad

F32 = mybir.dt.float32
F16 = mybir.dt.float16
BF16 = mybir.dt.bfloat16
U32 = mybir.dt.uint32
I32 = mybir.dt.int32

DOTS_ENGINE = os.environ.get("CMDP_DOTS", "vector")

_CACHED = {}


def _build_nc():
    nc = bacc.Bacc("TRN2", target_bir_lowering=False, debug=False,
                   num_devices=N_CORES)

    xT_d = nc.dram_tensor("xT", [128, KC * T], BF16, kind="ExternalInput")
    cbT_d = nc.dram_tensor("cbT", [128, KC * V], BF16, kind="ExternalInput")
    csqbc_d = nc.dram_tensor("csqbc", [128, V], F32, kind="ExternalInput")
    csqL_d = nc.dram_tensor("csqL", [2, 128], F16, kind="ExternalInput")
    csqR_d = nc.dram_tensor("csqR", [2, V], F16, kind="ExternalInput")
    cbe_d = nc.dram_tensor("cbe", [V, DE], F32, kind="ExternalInput")
    cbyb_d = nc.dram_tensor("cbyb", [V, D], BF16, kind="ExternalInput")
    w_d = nc.dram_tensor("w", [128, TT * 128], BF16, kind="ExternalInput")
    noise_d = nc.dram_tensor("noise", [T, D], F32, kind="ExternalInput")
    xn_d = nc.dram_tensor("xn", [128, TT * D], F32, kind="ExternalInput")
    mc_d = nc.dram_tensor("mc", [128, 64], F32, kind="ExternalInput")
    out_d = nc.dram_tensor("out", [T, D], F32, kind="ExternalOutput")

    with tile.TileContext(nc) as tc:
        with (
            tc.tile_pool(name="big", bufs=1) as big,
            tc.tile_pool(name="work", bufs=2) as work,
            tc.tile_pool(name="sc", bufs=2) as sc_pool,
            tc.tile_pool(name="ypool", bufs=4) as ypool,
            tc.tile_pool(name="io", bufs=3) as io,
            tc.tile_pool(name="ps_s", bufs=3, space="PSUM") as ps_s,
            tc.tile_pool(name="ps_m", bufs=1, space="PSUM") as ps_m,
        ):
            XTW = KC * 128   # xt columns per token tile
            VBW = KC * 512   # codebook columns per v-block
            # --- input staging, ordered so the PE can start ASAP ---
            csql = big.tile([2, 128], F16)
            csqr = big.tile([2, V], F16)
            nc.sync.dma_start(csql[:], csqL_d.ap())
            nc.sync.dma_start(csqr[:], csqR_d.ap())
            xt_t = []
            tl = big.tile([128, XTW], BF16, tag="xt0")
            nc.sync.dma_start(tl[:, 0:3 * 128], xT_d.ap()[:, 0:3 * 128])
            nc.sync.dma_start(tl[:, 3 * 128:XTW], xT_d.ap()[:, 3 * 128:XTW])
            xt_t.append(tl)
            cb_t = []
            tl = big.tile([128, VBW], BF16, tag="cbv0")
            for k in range(KC):
                nc.sync.dma_start(tl[:, k * 512:(k + 1) * 512],
                                  cbT_d.ap()[:, k * 512:(k + 1) * 512])
            cb_t.append(tl)
            csqbc = big.tile([128, V], F32)
            nc.sync.dma_start(csqbc[:, 0:1024], csqbc_d.ap()[:, 0:1024])
            for v in range(1, NV):
                tl = big.tile([128, VBW], BF16, tag=f"cbv{v}")
                nc.sync.dma_start(tl[:], cbT_d.ap()[:, v * VBW:(v + 1) * VBW])
                cb_t.append(tl)
                if v < 4:
                    nc.sync.dma_start(csqbc[:, v * 1024:(v + 1) * 1024],
                                      csqbc_d.ap()[:, v * 1024:(v + 1) * 1024])
            xn_t = []
            tl = big.tile([128, D], F32, tag="xn0")
            nc.sync.dma_start(tl[:], xn_d.ap()[:, 0:D])
            xn_t.append(tl)
            for t in range(1, TT):
                tl = big.tile([128, XTW], BF16, tag=f"xt{t}")
                nc.sync.dma_start(tl[:], xT_d.ap()[:, t * XTW:(t + 1) * XTW])
                xt_t.append(tl)
                tl = big.tile([128, D], F32, tag=f"xn{t}")
                nc.sync.dma_start(tl[:], xn_d.ap()[:, t * D:(t + 1) * D])
                xn_t.append(tl)
            w = big.tile([128, TT * 128], BF16)
            nc.sync.dma_start(w[:], w_d.ap())
            # merge constants for the last tile: cols 0:32 iota, 32:64
            # block offsets (1024*(c//8))
            mconst = big.tile([128, 64], F32)
            nc.sync.dma_start(mconst[:], mc_d.ap())
            # last tile's noise pre-staged in SBUF: its add runs on the (by
            # then idle) DVE instead of the ACT-drain -> accum-DMA chain
            nzlast = big.tile([128, 2 * D], F32)
            for a in range(2):
                tt_ = TT - 2 + a
                nc.sync.dma_start(nzlast[:, a * D:(a + 1) * D],
                                  noise_d.ap()[tt_ * 128:(tt_ + 1) * 128, :])

            ps_of = {}

            def emit_prefill(i):
                """ACT-prefill the csq bias for scoring block i (t>=1)."""
                vp = i % NVP
                ps = ps_s.tile([128, 1024], F32, tag="ps_score")
                nc.scalar.copy(out=ps[:],
                               in_=csqbc[:, vp * 1024:(vp + 1) * 1024])
                ps_of[i] = ps

            def emit_block(t, vp, scores):
                """12 bf16 matmuls for v-pair vp of tile t, drain to SBUF."""
                i = t * NVP + vp
                vsl = slice(vp * 1024, (vp + 1) * 1024)
                if t == 0:
                    # open the banks: start=True + 2-row fp16 csq matmul
                    ps = ps_s.tile([128, 1024], F32, tag="ps_score")
                    ps_of[i] = ps
                    for h in range(2):
                        v = vp * 2 + h
                        hs = slice(h * 512, (h + 1) * 512)
                        for k in range(KC):
                            nc.tensor.matmul(
                                ps[:, hs],
                                xt_t[t][:, k * 128:(k + 1) * 128],
                                cb_t[v][:, k * 512:(k + 1) * 512],
                                start=(k == 0), stop=False)
                        nc.tensor.matmul(ps[:, hs], csql[:],
                                         csqr[:, v * 512:(v + 1) * 512],
                                         start=False, stop=True)
                else:
                    ps = ps_of.pop(i)
                    for h in range(2):
                        v = vp * 2 + h
                        hs = slice(h * 512, (h + 1) * 512)
                        for k in range(KC):
                            nc.tensor.matmul(
                                ps[:, hs],
                                xt_t[t][:, k * 128:(k + 1) * 128],
                                cb_t[v][:, k * 512:(k + 1) * 512],
                                start=False, stop=(k == KC - 1),
                                skip_group_check=True)
                if i + 2 >= NVP and i + 2 < NBLK:
                    emit_prefill(i + 2)
                nc.scalar.copy(out=scores[:, vsl], in_=ps[:])

            def emit_cands_full(scores):
                mx = work.tile([128, 8], F32, tag="mx")
                idx = work.tile([128, 8], U32, tag="idx")
                nc.vector.max(mx[:], scores[:])
                nc.vector.max_index(idx[:], mx[:], scores[:])
                cand = []
                for j in range(2):
                    cj = work.tile([128, 1], I32, tag=f"cand{j}")
                    nc.vector.tensor_copy(cj[:], idx[:, j:j + 1])
                    cand.append(cj)
                return cand

            def emit_cands_merge(vals32, idxl):
                """global top-2 from 4 per-block top-8s (last tile)."""
                idxgf = work.tile([128, 32], F32, tag="idxgf")
                nc.vector.tensor_copy(idxgf[:], idxl[:])
                nc.vector.tensor_tensor(out=idxgf[:], in0=idxgf[:],
                                        in1=mconst[:, 32:64],
                                        op=mybir.AluOpType.add)
                mv8 = work.tile([128, 8], F32, tag="mv8")
                pos8 = work.tile([128, 8], U32, tag="pos8")
                nc.vector.max(mv8[:], vals32[:])
                nc.vector.max_index(pos8[:], mv8[:], vals32[:])
                cand = []
                for j in range(2):
                    posf = work.tile([128, 1], F32, tag=f"posf{j}")
                    nc.vector.tensor_copy(posf[:], pos8[:, j:j + 1])
                    m = work.tile([128, 32], F32, tag=f"m{j}")
                    nc.vector.tensor_scalar(
                        out=m[:], in0=mconst[:, 0:32], scalar1=posf[:, :1],
                        scalar2=None, op0=mybir.AluOpType.is_equal)
                    junk = work.tile([128, 32], F32, tag=f"junk{j}")
                    candf = work.tile([128, 1], F32, tag=f"candf{j}")
                    nc.vector.scalar_tensor_tensor(
                        out=junk[:], in0=m[:], scalar=1.0, in1=idxgf[:],
                        op0=mybir.AluOpType.bypass,
                        op1=mybir.AluOpType.mult, accum_out=candf[:])
                    cj = work.tile([128, 1], I32, tag=f"cand{j}")
                    nc.vector.tensor_copy(cj[:], candf[:])
                    cand.append(cj)
                return cand

            def emit_fixup(t, cand):
                """exact top-2 rescore -> gather bf16 y rows for the mix."""
                xn = xn_t[t][:]
                dot_eng = nc.gpsimd if DOTS_ENGINE == "gpsimd" else nc.vector
                sj = []
                for j in range(2):
                    g = work.tile([128, DE], F32, tag=f"g{j}")
                    nc.gpsimd.indirect_dma_start(
                        out=g[:], out_offset=None, in_=cbe_d.ap(),
                        in_offset=bass.IndirectOffsetOnAxis(
                            ap=cand[j][:, :1], axis=0))
                    # NB: tensor_tensor_reduce hard-faults TRN2 here;
                    # scalar_tensor_tensor with accum_out does not.
                    tmp = work.tile([128, D], F32, tag=f"rescore_tmp{j}")
                    dj = work.tile([128, 1], F32, tag=f"d{j}")
                    dot_eng.scalar_tensor_tensor(
                        out=tmp[:], in0=xn, scalar=1.0, in1=g[:, 0:D],
                        op0=mybir.AluOpType.bypass,
                        op1=mybir.AluOpType.mult, accum_out=dj[:])
                    s = work.tile([128, 1], F32, tag=f"s{j}")
                    # s = (dj * -2) + csq_cand
                    nc.vector.scalar_tensor_tensor(
                        out=s[:], in0=dj[:], scalar=-2.0, in1=g[:, D:D + 1],
                        op0=mybir.AluOpType.mult, op1=mybir.AluOpType.add)
                    sj.append(s)
                flip = work.tile([128, 1], I32, tag="flip")
                nc.vector.tensor_tensor(out=flip[:], in0=sj[1][:],
                                        in1=sj[0][:],
                                        op=mybir.AluOpType.is_lt)
                idx32 = work.tile([128, 1], I32, tag="idx32")
                nc.vector.tensor_copy(idx32[:], cand[0][:])
                nc.vector.copy_predicated(idx32[:], flip[:], cand[1][:])

                y = ypool.tile([128, D], BF16, tag="y")
                nc.gpsimd.indirect_dma_start(
                    out=y[:], out_offset=None, in_=cbyb_d.ap(),
                    in_offset=bass.IndirectOffsetOnAxis(ap=idx32[:, :1], axis=0))
                return y

            def emit_output(t, y):
                """bf16 mix matmul -> ACT drain -> noise accum-DMA -> store."""
                tsl = slice(t * 128, (t + 1) * 128)
                pm = ps_m.tile([128, D], F32, tag="pm")
                nc.tensor.matmul(pm[:, 0:512], w[:, tsl], y[:, 0:512],
                                 start=True, stop=True)
                nc.tensor.matmul(pm[:, 512:D], w[:, tsl], y[:, 512:D],
                                 start=True, stop=True)
                ob = io.tile([128, D], F32, tag="out")
                if t >= TT - 2:
                    nz = nzlast[:, (t - (TT - 2)) * D:(t - (TT - 2) + 1) * D]
                    nc.vector.tensor_add(ob[:], pm[:], nz)
                else:
                    nc.scalar.copy(out=ob[:], in_=pm[:])
                    # add DP noise inline in the DMA (SWDGE accumulate)
                    nc.gpsimd.dma_start(out=ob[:], in_=noise_d.ap()[tsl, :],
                                        accum_op=mybir.AluOpType.add)
                nc.sync.dma_start(out_d.ap()[tsl, :], ob[:])

            # 3-deep software pipeline: mix(t) runs ~3 tiles after score(t)
            # so the scan/fixup/gather chain never stalls the PE.
            PIPE = 3
            pending = []
            for t in range(TT):
                last = (t == TT - 1)
                scores = sc_pool.tile([128, V], F32, tag="scores")
                if last:
                    vals32 = work.tile([128, 32], F32, tag="vals32")
                    idxl = work.tile([128, 32], U32, tag="idxl")
                for vp in range(NVP):
                    emit_block(t, vp, scores)
                    if last:
                        vsl = slice(vp * 1024, (vp + 1) * 1024)
                        ssl = slice(vp * 8, (vp + 1) * 8)
                        nc.vector.max(vals32[:, ssl], scores[:, vsl])
                        nc.vector.max_index(idxl[:, ssl], vals32[:, ssl],
                                            scores[:, vsl])
                if last:
                    cand = emit_cands_merge(vals32, idxl)
                else:
                    cand = emit_cands_full(scores)
                y = emit_fixup(t, cand)
                pending.append((t, y))
                if len(pending) > PIPE:
                    emit_output(*pending.pop(0))
            for item in pending:
                emit_output(*item)

    nc.compile()
    return nc


def _prep_inputs(base_embeddings, codebook, rand_idx, noise):
    """Build the 8 per-core input maps (all host-side numpy)."""
    import ml_dtypes
    x = np.ascontiguousarray(base_embeddings, dtype=np.float32)
    cb = np.ascontiguousarray(codebook, dtype=np.float32)
    ridx = np.asarray(rand_idx)
    nz = np.asarray(noise, dtype=np.float32)

    csq = (cb * cb).sum(-1, dtype=np.float32)              # [V]
    cbe = np.zeros((V, DE), np.float32)
    cbe[:, :D] = cb
    cbe[:, D] = csq
    csqc = (csq - CSQ_CENTER).astype(np.float32)
    # fp32 broadcast bias for ACT prefill (tiles >= 1)
    csqbc = np.ascontiguousarray(
        np.broadcast_to(-csqc[None, :], (128, V)).astype(np.float32))
    # fp16 hi/lo pair for tile 0's in-matmul csq bias
    r1 = csqc.astype(np.float16)
    r2 = (csqc - r1.astype(np.float32)).astype(np.float16)
    csqR = np.ascontiguousarray(np.stack([r1, r2]))        # [2, V] fp16
    csqL = np.full((2, 128), -1.0, np.float16)
    cbyb = cb.astype(ml_dtypes.bfloat16)                   # [V, D] bf16

    # merge constants: [128, 64] f32; cols 0:32 iota, 32:64 1024*(c//8)
    mc = np.zeros((128, 64), np.float32)
    mc[:, 0:32] = np.arange(32, dtype=np.float32)[None, :]
    mc[:, 32:64] = (1024.0 * (np.arange(32) // 8)).astype(np.float32)[None, :]
    mc = np.ascontiguousarray(mc)

    # pre-tile [D, V] -> [128, (v, k, 512)] v-block-major layout
    cbT = cb.T.reshape(KC, 128, NV, 512).transpose(1, 2, 0, 3).reshape(128, KC * V)
    cbT = np.ascontiguousarray(cbT).astype(ml_dtypes.bfloat16)

    shared = {"cbe": cbe, "csqbc": csqbc, "csqL": csqL, "csqR": csqR,
              "cbyb": cbyb, "mc": mc, "cbT": cbT}

    in_maps = []
    for c in range(N_CORES):
        ssl = slice(c * SS, (c + 1) * SS)
        # tokens t = s_local*16 + b
        xc = x[:, ssl, :].transpose(1, 0, 2).reshape(T, D)
        xT2 = (2.0 * xc).T                                 # [D, T] fp32
        # pre-tile [D, T] -> [128, (t, k, 128)] tile-major layout
        xT2 = np.ascontiguousarray(
            xT2.reshape(KC, 128, TT, 128).transpose(1, 2, 0, 3).reshape(128, KC * T))
        nzc = np.ascontiguousarray(
            DP_EPSILON * nz[:, ssl, :].transpose(1, 0, 2).reshape(T, D))
        rc = ridx[:, ssl, :]                               # [B, SS, K]
        wm = np.zeros((TT, 128, 128), np.float32)
        for tt in range(TT):
            for g in range(8):
                s_local = tt * 8 + g
                r = rc[:, s_local, :]                      # [B, K] in [0,B)
                cnt = np.zeros((B, B), np.float32)         # [dst=b, src]
                for bdst in range(B):
                    np.add.at(cnt[bdst], r[bdst], 1.0)
                wm[tt, g * 16:(g + 1) * 16, g * 16:(g + 1) * 16] = cnt.T / K
        wm_t = np.ascontiguousarray(
            wm.transpose(1, 0, 2).reshape(128, TT * 128)).astype(ml_dtypes.bfloat16)
        m = {"w": wm_t, "noise": nzc, **shared,
             "xT": xT2.astype(ml_dtypes.bfloat16),
             "xn": np.ascontiguousarray(
                 xc.reshape(TT, 128, D).transpose(1, 0, 2).reshape(128, TT * D))}
        in_maps.append(m)
    return in_maps


def kernel(base_embeddings, codebook, rand_idx, noise, _results_out=None):
    if "nc" not in _CACHED:
        _CACHED["nc"] = _build_nc()
    nc = _CACHED["nc"]
    in_maps = _prep_inputs(base_embeddings, codebook, rand_idx, noise)
    res = run_bass_kernel_spmd(nc, in_maps, list(range(N_CORES)))
    if _results_out is not None:
        _results_out.append(res)
    outs = []
    for c in range(N_CORES):
        oc = res.results[c]["out"].reshape(SS, B, D).transpose(1, 0, 2)
        outs.append(oc)
    return np.ascontiguousarray(np.concatenate(outs, axis=1))


# revision 10
# speedup vs baseline: 1.1490x; 1.0671x over previous
"""Trainium2 Bass kernel for nn_CMDPEncoder (VQ codebook quantize + random
batch-mix dequantize + DP noise).

Reference semantics:
    dots = einsum('bsd,vd->bsv', base, codebook)
    qi   = argmin_v(csq[v] - 2*dots)                  # [B,S]
    codes[b,s,j] = qi[rand_idx[b,s,j], s]
    out  = mean_j codebook[codes] + 0.1*noise

Sharding: split the sequence dim S across the 8 cores (64 positions each).
The rand_idx mixing crosses only the batch dim at fixed s, so with S-sharding
every core's mixing is fully local (no collectives).  Tokens are laid out
s-major (t = s_local*16 + b) so each 128-token tile holds 8 complete
s-groups of 16 batches, and the mix becomes a block-diagonal [128,128]
matmul with host-precomputed weights (counts/4 from rand_idx).

v4 design:
  - scores accumulate in [128,1024] (2-bank) PSUM tiles; csq bias comes in
    via ACT-engine PSUM prefill (fp32 exact) + all-start=False matmuls.
    A start=False matmul on a never-opened bank does not accumulate
    correctly on HW (v2 post-mortem), so 6 throwaway start=True matmuls
    open all 6 score banks at kernel start.
  - prefills are issued 2 blocks ahead so the in-order ACT engine finishes
    them under the previous blocks' matmuls instead of stalling the PE.
  - tiles 0 and 1 are interleaved per v-pair at the head so the PE does two
    tiles of work per codebook v-block DMA arrival (the 6.3 MB cbT stream
    can't otherwise keep up with one tile's consumption rate).
  - per-v-pair incremental MAX8 for every tile; a tiny [128,32] merge
    produces the global top-8 values, and one full FIND_INDEX8 recovers
    positions.  The last tile also does FIND incrementally per v-pair and
    merges indices via an iota-select, shortening the serial tail.
  - mix matmul in bf16 (w is exact in bf16; y gathered from a bf16 copy of
    the codebook).  Exact top-2 rescore fixup as before (DVE dots).
"""

import os
import sys

for p in ("/opt/trn_rl_repo",):
    if p not in sys.path:
        sys.path.insert(0, p)

import numpy as np

import concourse.bacc as bacc
import concourse.bass as bass
import concourse.mybir as mybir
import concourse.tile as tile
from concourse.bass_utils import run_bass_kernel_spmd

B, S, D, V, K = 16, 512, 768, 4096, 4
N_CORES = 8
SS = S // N_CORES            # 64 sequence positions per core
T = SS * B                   # 1024 tokens per core, t = s_local*16 + b
TT = T // 128                # 8 token tiles per core
KC = D // 128                # 6 contraction chunks
NV = V // 512                # 8 V-tiles
NVP = V // 1024              # 4 V-pairs (2-bank PSUM tiles)
DP_EPSILON = 0.1
CSQ_CENTER = 768.0
DE = 776                     # padded cb_ext row: 768 cb + 1 csq + 7 pad

F32 = mybir.dt.float32
F16 = mybir.dt.float16
BF16 = mybir.dt.bfloat16
U32 = mybir.dt.uint32
I32 = mybir.dt.int32

_CACHED = {}


def _build_nc():
    nc = bacc.Bacc("TRN2", target_bir_lowering=False, debug=False,
                   num_devices=N_CORES)

    xT_d = nc.dram_tensor("xT", [128, KC * T], BF16, kind="ExternalInput")
    cbT_d = nc.dram_tensor("cbT", [128, KC * V], BF16, kind="ExternalInput")
    csqbc_d = nc.dram_tensor("csqbc", [128, V], F32, kind="ExternalInput")
    csqL_d = nc.dram_tensor("csqL", [2, 128], F16, kind="ExternalInput")
    csqR_d = nc.dram_tensor("csqR", [2, V], F16, kind="ExternalInput")
    cbe_d = nc.dram_tensor("cbe", [V, DE], F32, kind="ExternalInput")
    cbyb_d = nc.dram_tensor("cbyb", [V, D], BF16, kind="ExternalInput")
    w_d = nc.dram_tensor("w", [128, TT * 128], BF16, kind="ExternalInput")
    noise_d = nc.dram_tensor("noise", [T, D], F32, kind="ExternalInput")
    xn_d = nc.dram_tensor("xn", [128, TT * D], F32, kind="ExternalInput")
    mc_d = nc.dram_tensor("mc", [128, 64], F32, kind="ExternalInput")
    out_d = nc.dram_tensor("out", [T, D], F32, kind="ExternalOutput")

    with tile.TileContext(nc) as tc:
        with (
            tc.tile_pool(name="big", bufs=1) as big,
            tc.tile_pool(name="work", bufs=2) as work,
            tc.tile_pool(name="sc", bufs=3) as sc_pool,
            tc.tile_pool(name="ypool", bufs=4) as ypool,
            tc.tile_pool(name="io", bufs=3) as io,
            tc.tile_pool(name="ps_s", bufs=3, space="PSUM") as ps_s,
            tc.tile_pool(name="ps_m", bufs=1, space="PSUM") as ps_m,
        ):
            XTW = KC * 128   # xt columns per token tile
            VBW = KC * 512   # codebook columns per v-block
            # --- input staging, ordered so the PE can start ASAP and the
            # cbT stream stays ahead of the (tile0,tile1)-interleaved head ---
            csql = big.tile([2, 128], F16)
            csqr = big.tile([2, V], F16)
            nc.sync.dma_start(csql[:], csqL_d.ap())
            nc.sync.dma_start(csqr[:], csqR_d.ap())
            csqbc = big.tile([128, V], F32)
            nc.sync.dma_start(csqbc[:, 0:1024], csqbc_d.ap()[:, 0:1024])
            xt_t = [None] * TT
            for t in (0, 1):
                tl = big.tile([128, XTW], BF16, tag=f"xt{t}")
                nc.sync.dma_start(tl[:, 0:3 * 128], xT_d.ap()[:, t * XTW:t * XTW + 3 * 128])
                nc.sync.dma_start(tl[:, 3 * 128:XTW], xT_d.ap()[:, t * XTW + 3 * 128:(t + 1) * XTW])
                xt_t[t] = tl
            cb_t = []
            tl = big.tile([128, VBW], BF16, tag="cbv0")
            for k in range(KC):
                nc.sync.dma_start(tl[:, k * 512:(k + 1) * 512],
                                  cbT_d.ap()[:, k * 512:(k + 1) * 512])
            cb_t.append(tl)
            tl = big.tile([128, VBW], BF16, tag="cbv1")
            nc.sync.dma_start(tl[:], cbT_d.ap()[:, VBW:2 * VBW])
            cb_t.append(tl)
            nc.sync.dma_start(csqbc[:, 1024:2048], csqbc_d.ap()[:, 1024:2048])
            for v in (2, 3):
                tl = big.tile([128, VBW], BF16, tag=f"cbv{v}")
                nc.sync.dma_start(tl[:], cbT_d.ap()[:, v * VBW:(v + 1) * VBW])
                cb_t.append(tl)
            nc.sync.dma_start(csqbc[:, 2048:3072], csqbc_d.ap()[:, 2048:3072])
            for v in (4, 5):
                tl = big.tile([128, VBW], BF16, tag=f"cbv{v}")
                nc.sync.dma_start(tl[:], cbT_d.ap()[:, v * VBW:(v + 1) * VBW])
                cb_t.append(tl)
            nc.sync.dma_start(csqbc[:, 3072:4096], csqbc_d.ap()[:, 3072:4096])
            for v in (6, 7):
                tl = big.tile([128, VBW], BF16, tag=f"cbv{v}")
                nc.sync.dma_start(tl[:], cbT_d.ap()[:, v * VBW:(v + 1) * VBW])
                cb_t.append(tl)
            xn_t = [None] * TT
            for t in (0, 1):
                tl = big.tile([128, D], F32, tag=f"xn{t}")
                nc.sync.dma_start(tl[:], xn_d.ap()[:, t * D:(t + 1) * D])
                xn_t[t] = tl
            for t in range(2, TT):
                tl = big.tile([128, XTW], BF16, tag=f"xt{t}")
                nc.sync.dma_start(tl[:], xT_d.ap()[:, t * XTW:(t + 1) * XTW])
                xt_t[t] = tl
                tl = big.tile([128, D], F32, tag=f"xn{t}")
                nc.sync.dma_start(tl[:], xn_d.ap()[:, t * D:(t + 1) * D])
                xn_t[t] = tl
            w = big.tile([128, TT * 128], BF16)
            nc.sync.dma_start(w[:], w_d.ap())
            # merge constants for the last tile: cols 0:32 iota, 32:64
            # block offsets (1024*(c//8))
            mconst = big.tile([128, 64], F32)
            nc.sync.dma_start(mconst[:], mc_d.ap())
            # last 2 tiles' noise pre-staged in SBUF: their adds run on
            # GpSimd instead of the ACT-drain -> accum-DMA chain
            nzlast = big.tile([128, 2 * D], F32)
            for a in range(2):
                tt_ = TT - 2 + a
                nc.sync.dma_start(nzlast[:, a * D:(a + 1) * D],
                                  noise_d.ap()[tt_ * 128:(tt_ + 1) * 128, :])

            # block schedule: tiles 0/1 interleaved pairwise at the head so
            # the PE does two tiles of work per codebook v-block arrival,
            # then tiles 2..7 sequential.
            sched = [(0, 0), (1, 0), (0, 1), (1, 1), (0, 2), (1, 2),
                     (0, 3), (1, 3)]
            for t in range(2, TT):
                sched += [(t, vp) for vp in range(NVP)]

            # open all 6 score PSUM banks with throwaway start=True matmuls
            # (values are overwritten by the first prefill of each buffer)
            for _ in range(3):
                psd = ps_s.tile([128, 1024], F32, tag="ps_score")
                for h in range(2):
                    nc.tensor.matmul(psd[:, h * 512:(h + 1) * 512],
                                     csql[:], csqr[:, 0:512],
                                     start=True, stop=True)

            ps_of = {}

            def emit_prefill(j):
                """ACT-prefill the csq bias for schedule slot j."""
                _, vp = sched[j]
                ps = ps_s.tile([128, 1024], F32, tag="ps_score")
                nc.scalar.copy(out=ps[:],
                               in_=csqbc[:, vp * 1024:(vp + 1) * 1024])
                ps_of[j] = ps

            emit_prefill(0)
            emit_prefill(1)

            def emit_block(j, scores):
                """12 bf16 matmuls for schedule slot j, drain to SBUF."""
                t, vp = sched[j]
                vsl = slice(vp * 1024, (vp + 1) * 1024)
                ps = ps_of.pop(j)
                for h in range(2):
                    v = vp * 2 + h
                    hs = slice(h * 512, (h + 1) * 512)
                    for k in range(KC):
                        nc.tensor.matmul(
                            ps[:, hs],
                            xt_t[t][:, k * 128:(k + 1) * 128],
                            cb_t[v][:, k * 512:(k + 1) * 512],
                            start=False, stop=(k == KC - 1),
                            skip_group_check=True)
                if j + 2 < len(sched):
                    emit_prefill(j + 2)
                nc.scalar.copy(out=scores[:, vsl], in_=ps[:])

            def emit_cands(scores, vals32, idxl):
                """global top-2 candidate indices [128,1] i32 each."""
                mv8 = work.tile([128, 8], F32, tag="mv8")
                nc.vector.max(mv8[:], vals32[:])
                if idxl is None:
                    idx = work.tile([128, 8], U32, tag="idx")
                    nc.vector.max_index(idx[:], mv8[:], scores[:])
                    cand = []
                    for jj in range(2):
                        cj = work.tile([128, 1], I32, tag=f"cand{jj}")
                        nc.vector.tensor_copy(cj[:], idx[:, jj:jj + 1])
                        cand.append(cj)
                    return cand
                # last tile: merge the 4 per-block top-8 indices instead of
                # a full-array FIND (shorter serial tail)
                idxgf = work.tile([128, 32], F32, tag="idxgf")
                nc.vector.tensor_copy(idxgf[:], idxl[:])
                nc.vector.tensor_tensor(out=idxgf[:], in0=idxgf[:],
                                        in1=mconst[:, 32:64],
                                        op=mybir.AluOpType.add)
                pos8 = work.tile([128, 8], U32, tag="pos8")
                nc.vector.max_index(pos8[:], mv8[:], vals32[:])
                cand = []
                for jj in range(2):
                    posf = work.tile([128, 1], F32, tag=f"posf{jj}")
                    nc.vector.tensor_copy(posf[:], pos8[:, jj:jj + 1])
                    m = work.tile([128, 32], F32, tag=f"m{jj}")
                    nc.vector.tensor_scalar(
                        out=m[:], in0=mconst[:, 0:32], scalar1=posf[:, :1],
                        scalar2=None, op0=mybir.AluOpType.is_equal)
                    junk = work.tile([128, 32], F32, tag=f"junk{jj}")
                    candf = work.tile([128, 1], F32, tag=f"candf{jj}")
                    nc.vector.scalar_tensor_tensor(
                        out=junk[:], in0=m[:], scalar=1.0, in1=idxgf[:],
                        op0=mybir.AluOpType.bypass,
                        op1=mybir.AluOpType.mult, accum_out=candf[:])
                    cj = work.tile([128, 1], I32, tag=f"cand{jj}")
                    nc.vector.tensor_copy(cj[:], candf[:])
                    cand.append(cj)
                return cand

            def emit_fixup(t, cand):
                """exact top-2 rescore -> gather bf16 y rows for the mix."""
                xn = xn_t[t][:]
                sj = []
                for jj in range(2):
                    g = work.tile([128, DE], F32, tag=f"g{jj}")
                    nc.gpsimd.indirect_dma_start(
                        out=g[:], out_offset=None, in_=cbe_d.ap(),
                        in_offset=bass.IndirectOffsetOnAxis(
                            ap=cand[jj][:, :1], axis=0))
                    # NB: tensor_tensor_reduce hard-faults TRN2 here;
                    # scalar_tensor_tensor with accum_out does not.
                    tmp = work.tile([128, D], F32, tag=f"rescore_tmp{jj}")
                    dj = work.tile([128, 1], F32, tag=f"d{jj}")
                    nc.vector.scalar_tensor_tensor(
                        out=tmp[:], in0=xn, scalar=1.0, in1=g[:, 0:D],
                        op0=mybir.AluOpType.bypass,
                        op1=mybir.AluOpType.mult, accum_out=dj[:])
                    s = work.tile([128, 1], F32, tag=f"s{jj}")
                    # s = (dj * -2) + csq_cand
                    nc.vector.scalar_tensor_tensor(
                        out=s[:], in0=dj[:], scalar=-2.0, in1=g[:, D:D + 1],
                        op0=mybir.AluOpType.mult, op1=mybir.AluOpType.add)
                    sj.append(s)
                flip = work.tile([128, 1], I32, tag="flip")
                nc.vector.tensor_tensor(out=flip[:], in0=sj[1][:],
                                        in1=sj[0][:],
                                        op=mybir.AluOpType.is_lt)
                idx32 = work.tile([128, 1], I32, tag="idx32")
                nc.vector.tensor_copy(idx32[:], cand[0][:])
                nc.vector.copy_predicated(idx32[:], flip[:], cand[1][:])

                y = ypool.tile([128, D], BF16, tag="y")
                nc.gpsimd.indirect_dma_start(
                    out=y[:], out_offset=None, in_=cbyb_d.ap(),
                    in_offset=bass.IndirectOffsetOnAxis(ap=idx32[:, :1], axis=0))
                return y

            def emit_output(t, y):
                """bf16 mix matmul -> ACT drain -> noise accum-DMA -> store."""
                tsl = slice(t * 128, (t + 1) * 128)
                pm = ps_m.tile([128, D], F32, tag="pm")
                nc.tensor.matmul(pm[:, 0:512], w[:, tsl], y[:, 0:512],
                                 start=True, stop=True)
                nc.tensor.matmul(pm[:, 512:D], w[:, tsl], y[:, 512:D],
                                 start=True, stop=True)
                ob = io.tile([128, D], F32, tag="out")
                if t >= TT - 2:
                    nz = nzlast[:, (t - (TT - 2)) * D:(t - (TT - 2) + 1) * D]
                    nc.vector.tensor_add(ob[:], pm[:], nz)
                else:
                    nc.scalar.copy(out=ob[:], in_=pm[:])
                    # add DP noise inline in the DMA (SWDGE accumulate)
                    nc.gpsimd.dma_start(out=ob[:], in_=noise_d.ap()[tsl, :],
                                        accum_op=mybir.AluOpType.add)
                nc.sync.dma_start(out_d.ap()[tsl, :], ob[:])

            # 3-deep software pipeline: mix(t) runs ~3 tiles after score(t)
            # so the scan/fixup/gather chain never stalls the PE.
            PIPE = 3
            pending = []
            scores_of, vals_of, idxl_of, nvp_done = {}, {}, {}, {}
            for j in range(len(sched)):
                t, vp = sched[j]
                last = (t == TT - 1)
                if t not in scores_of:
                    scores_of[t] = sc_pool.tile([128, V], F32, tag="scores", name=f"scores{t}")
                    vals_of[t] = work.tile([128, 32], F32, tag="vals32", name=f"vals32_{t}")
                    idxl_of[t] = (work.tile([128, 32], U32, tag="idxl", name=f"idxl{t}")
                                  if last else None)
                    nvp_done[t] = 0
                scores = scores_of[t]
                emit_block(j, scores)
                vsl = slice(vp * 1024, (vp + 1) * 1024)
                ssl = slice(vp * 8, (vp + 1) * 8)
                nc.vector.max(vals_of[t][:, ssl], scores[:, vsl])
                if last:
                    nc.vector.max_index(idxl_of[t][:, ssl],
                                        vals_of[t][:, ssl], scores[:, vsl])
                nvp_done[t] += 1
                if nvp_done[t] == NVP:
                    cand = emit_cands(scores, vals_of[t], idxl_of[t])
                    y = emit_fixup(t, cand)
                    pending.append((t, y))
                    if len(pending) > PIPE:
                        emit_output(*pending.pop(0))
            for item in pending:
                emit_output(*item)

    nc.compile()
    return nc


def _prep_inputs(base_embeddings, codebook, rand_idx, noise):
    """Build the 8 per-core input maps (all host-side numpy)."""
    import ml_dtypes
    x = np.ascontiguousarray(base_embeddings, dtype=np.float32)
    cb = np.ascontiguousarray(codebook, dtype=np.float32)
    ridx = np.asarray(rand_idx)
    nz = np.asarray(noise, dtype=np.float32)

    csq = (cb * cb).sum(-1, dtype=np.float32)              # [V]
    cbe = np.zeros((V, DE), np.float32)
    cbe[:, :D] = cb
    cbe[:, D] = csq
    csqc = (csq - CSQ_CENTER).astype(np.float32)
    # fp32 broadcast bias for ACT prefill (tiles >= 1)
    csqbc = np.ascontiguousarray(
        np.broadcast_to(-csqc[None, :], (128, V)).astype(np.float32))
    # fp16 hi/lo pair for tile 0's in-matmul csq bias
    r1 = csqc.astype(np.float16)
    r2 = (csqc - r1.astype(np.float32)).astype(np.float16)
    csqR = np.ascontiguousarray(np.stack([r1, r2]))        # [2, V] fp16
    csqL = np.full((2, 128), -1.0, np.float16)
    cbyb = cb.astype(ml_dtypes.bfloat16)                   # [V, D] bf16

    # merge constants: [128, 64] f32; cols 0:32 iota, 32:64 1024*(c//8)
    mc = np.zeros((128, 64), np.float32)
    mc[:, 0:32] = np.arange(32, dtype=np.float32)[None, :]
    mc[:, 32:64] = (1024.0 * (np.arange(32) // 8)).astype(np.float32)[None, :]
    mc = np.ascontiguousarray(mc)

    # pre-tile [D, V] -> [128, (v, k, 512)] v-block-major layout
    cbT = cb.T.reshape(KC, 128, NV, 512).transpose(1, 2, 0, 3).reshape(128, KC * V)
    cbT = np.ascontiguousarray(cbT).astype(ml_dtypes.bfloat16)

    shared = {"cbe": cbe, "csqbc": csqbc, "csqL": csqL, "csqR": csqR,
              "cbyb": cbyb, "mc": mc, "cbT": cbT}

    in_maps = []
    for c in range(N_CORES):
        ssl = slice(c * SS, (c + 1) * SS)
        # tokens t = s_local*16 + b
        xc = x[:, ssl, :].transpose(1, 0, 2).reshape(T, D)
        xT2 = (2.0 * xc).T                                 # [D, T] fp32
        # pre-tile [D, T] -> [128, (t, k, 128)] tile-major layout
        xT2 = np.ascontiguousarray(
            xT2.reshape(KC, 128, TT, 128).transpose(1, 2, 0, 3).reshape(128, KC * T))
        nzc = np.ascontiguousarray(
            DP_EPSILON * nz[:, ssl, :].transpose(1, 0, 2).reshape(T, D))
        rc = ridx[:, ssl, :]                               # [B, SS, K]
        wm = np.zeros((TT, 128, 128), np.float32)
        for tt in range(TT):
            for g in range(8):
                s_local = tt * 8 + g
                r = rc[:, s_local, :]                      # [B, K] in [0,B)
                cnt = np.zeros((B, B), np.float32)         # [dst=b, src]
                for bdst in range(B):
                    np.add.at(cnt[bdst], r[bdst], 1.0)
                wm[tt, g * 16:(g + 1) * 16, g * 16:(g + 1) * 16] = cnt.T / K
        wm_t = np.ascontiguousarray(
            wm.transpose(1, 0, 2).reshape(128, TT * 128)).astype(ml_dtypes.bfloat16)
        m = {"w": wm_t, "noise": nzc, **shared,
             "xT": xT2.astype(ml_dtypes.bfloat16),
             "xn": np.ascontiguousarray(
                 xc.reshape(TT, 128, D).transpose(1, 0, 2).reshape(128, TT * D))}
        in_maps.append(m)
    return in_maps


def kernel(base_embeddings, codebook, rand_idx, noise, _results_out=None):
    if "nc" not in _CACHED:
        _CACHED["nc"] = _build_nc()
    nc = _CACHED["nc"]
    in_maps = _prep_inputs(base_embeddings, codebook, rand_idx, noise)
    res = run_bass_kernel_spmd(nc, in_maps, list(range(N_CORES)))
    if _results_out is not None:
        _results_out.append(res)
    outs = []
    for c in range(N_CORES):
        oc = res.results[c]["out"].reshape(SS, B, D).transpose(1, 0, 2)
        outs.append(oc)
    return np.ascontiguousarray(np.concatenate(outs, axis=1))
